# revision 34
# baseline (speedup 1.0000x reference)
"""Trainium2 Bass kernel for nn_Mean_2px_Pad2d.

Full input x: [128, 96, 64, 64] f32.  Output: [128, 96, 66, 66] f32:
  - interior = x
  - borders  = edge-replicate pad, with top/bot rows (cols 1..64) and
    left/right cols (rows 1..64) overwritten by 2-pixel boundary means
  - patches on the image boundary (P=4 grid, 16 patches per image) get
    their outer border row/col zeroed (full 66 length incl. corners)

Sharding: batch 128 = 8 images x 16 patches; one image (16 consecutive
batch entries) per NeuronCore -> identical SPMD program on 8 cores.

Perf design (measured on HW, not theorized):
  - Device output is bf16, upcast to f32 on the host: rounding happens
    AFTER the f32 boundary means so max rel err ~2^-9, far inside the
    2e-2 gate, and it halves store-side HBM traffic.
  - ALL x/y DMA runs on ONE queue (SP HWDGE), FIFO-interleaved with a
    one-chunk lag: L0,L1,S0,L2,S1,...  Measured HW behavior: with
    separate load/store queues, arbitration starves the compute-gated
    store queue whenever the load queue has descriptors ready, building
    a store backlog that stalls the pipeline through buffer
    backpressure and then drains in a slow stuttering tail. FIFO on a
    single queue enforces the byte ratio exactly, one queue sustains
    ~424 GB/s (microbenchmarked; a 2nd adds nothing, a 3rd costs ~15%),
    and per-core the 16 DMA engines run ~100% busy. All 8 cores
    together saturate chip HBM at ~3 TB/s.
  - K=2 CONSECUTIVE channel-images per partition: they are contiguous
    in both x and y, so per-partition DMA runs are 32 KiB loads and
    17424 B stores. Per-engine DMA rate saturates (~27 B/ns) at 32 KiB
    runs -- K=3's 48 KiB runs measured NO faster and its bigger tiles
    force shallower buffering; K=2 with tin x3 / tout x4 buffers won.
  - 96 channel-images per patch divides by K, so an image group never
    straddles a patch boundary; boundary-patch zeroing is a per-
    partition 0/1 mask multiply (patch intervals in partition space do
    not start on the 0/32/64/96 bases compute ops require). The masks
    ([128] x 6 mega-tiles x 4 edges) are a host-precomputed constant
    table DMA'd once at start on the otherwise-idle ACT queue.
  - Tail structure: a tiny 8-row chunk split off tile 0 has its store
    emitted as the queue's LAST item (computed ~90 us earlier, so the
    queue never ends waiting on compute -- the tile scheduler can
    reorder ready stores ahead of the last load, leaving only
    compute-gated work at the end otherwise), and the last tile is
    split 56+8 so the final compute on the critical path is ~1 us.
"""

import sys

import numpy as np

try:
    import concourse.bass as bass
except ImportError:
    sys.path.insert(0, "/opt/trn_rl_repo")
    import concourse.bass as bass

import concourse.mybir as mybir
import concourse.tile as tile
from concourse.bass_utils import run_bass_kernel_spmd

F32 = mybir.dt.float32
BF16 = mybir.dt.bfloat16

# Per-core shard shapes (hardcoded; full batch 128 / 8 cores).
BSH = 16          # batch entries (patches) per core = one image
C = 96            # channels
H = W = 64
HO = WO = 66      # padded output
G = BSH * C       # 1536 channel-images per core
PT = 128          # partitions per tile
K = 2             # channel-images per partition: 32 KiB load runs,
                  # 17424 B store runs
NT = G // (PT * K)  # 6 mega-tiles
NCORES = 8

TOP, BOT, LEFT, RIGHT = range(4)


def _emit_load_compute(nc, pool, xv, yv, zm, t, r0, n, tag=""):
    """Rows [r0, r0+n) of all K images of a [128, K-image] mega-tile:
    input rows r0..r0+n-1 -> output rows (r0+1)..(r0+n), plus the top
    border row if r0==0 and the bottom border row if r0+n==H, plus
    left/right border cols and boundary-patch zero masks. Returns the
    finished tout for a later (lagged) store emission."""
    g0 = t * PT                                            # image-pair index
    first = r0 == 0
    last = r0 + n == H
    orows = n + (1 if first else 0) + (1 if last else 0)   # output rows
    out_r0 = 0 if first else r0 + 1                        # global out row
    i0 = 1 if first else 0                                 # local 1st interior

    if tag:
        # Dedicated single-buffer slot: its store is held back to the very
        # end of the queue, so it must not block main-tag slot rotation.
        tin_flat = pool.tile([PT, K * n * W], F32,
                             tag=f"tin{tag}", name="tin_flat", bufs=1)
        tout_flat = pool.tile([PT, K * orows * WO], BF16,
                              tag=f"tout{tag}", name="tout_flat", bufs=1)
        tin = tin_flat.rearrange("p (k h w) -> p k h w", k=K, h=n, w=W)
        tout = tout_flat.rearrange("p (k h w) -> p k h w", k=K, h=orows, w=WO)
    else:
        # tin needs (loading + computing) slots, tout additionally covers
        # draining stores; one extra each absorbs scheduler jitter.
        tin_flat = pool.tile([PT, K * H * W], F32, tag="tin",
                             name="tin_flat", bufs=3)
        tout_flat = pool.tile([PT, K * HO * WO], BF16, tag="tout",
                              name="tout_flat", bufs=4)
        tin = tin_flat.rearrange(
            "p (k h w) -> p k h w", k=K, h=H, w=W)[:, :, :n, :]
        tout = tout_flat.rearrange(
            "p (k h w) -> p k h w", k=K, h=HO, w=WO)[:, :, :orows, :]

    # Full-height chunks DMA through the flat 2D view: one contiguous
    # 32 KiB (load) / 17424 B (store) run per partition. 4D APs do not
    # coalesce the image dim and would split each run K ways.
    if n == H:
        nc.sync.dma_start(
            out=tin_flat[:],
            in_=xv[g0:g0 + PT, :, :, :].rearrange("g k h w -> g (k h w)"))
    else:
        nc.sync.dma_start(out=tin[:], in_=xv[g0:g0 + PT, :, r0:r0 + n, :])

    # Dummy first write to tout (overwritten below): absorbs the slot-reuse
    # WAR wait so no later compute op carries two semaphore waits (TRN2
    # codegen allows a single sync-wait per instruction).
    nc.vector.memset(tout[:, 0, 0, 0:WO:WO - 1], 0.0)

    # Interior rows
    nc.vector.tensor_copy(tout[:, :, i0:i0 + n, 1:W + 1], tin[:])

    # Border rows (2-px means) + corners (edge-replicate)
    for br, (ra, rb) in (
        ([(0, (0, 1))] if first else []) +
        ([(orows - 1, (n - 2, n - 1))] if last else [])
    ):
        nc.vector.tensor_add(
            tout[:, :, br, 1:W + 1], tin[:, :, ra, :], tin[:, :, rb, :])
        nc.vector.tensor_scalar_mul(
            tout[:, :, br, 1:W + 1], tout[:, :, br, 1:W + 1], 0.5)
        rc = 0 if br == 0 else n - 1
        nc.vector.tensor_copy(
            tout[:, :, br, 0:WO:WO - 1], tin[:, :, rc, 0:W:W - 1])

    # Left+right border cols for this chunk's interior rows
    nc.vector.tensor_add(
        tout[:, :, i0:i0 + n, 0:WO:WO - 1],
        tin[:, :, :, 0:W:W - 2],
        tin[:, :, :, 1:W:W - 2],
    )
    nc.vector.tensor_scalar_mul(
        tout[:, :, i0:i0 + n, 0:WO:WO - 1],
        tout[:, :, i0:i0 + n, 0:WO:WO - 1], 0.5,
    )

    # Zero the outer border of boundary patches: multiply by per-partition
    # 0/1 masks (all K images of a partition share one patch, see header).
    if first:
        nc.vector.tensor_scalar_mul(
            tout[:, :, 0, :], tout[:, :, 0, :], zm[:, t, TOP:TOP + 1])
    if last:
        nc.vector.tensor_scalar_mul(
            tout[:, :, orows - 1, :], tout[:, :, orows - 1, :], zm[:, t, BOT:BOT + 1])
    nc.vector.tensor_scalar_mul(
        tout[:, :, :, 0], tout[:, :, :, 0], zm[:, t, LEFT:LEFT + 1])
    nc.vector.tensor_scalar_mul(
        tout[:, :, :, WO - 1], tout[:, :, :, WO - 1], zm[:, t, RIGHT:RIGHT + 1])

    return tout_flat, tout, g0, out_r0, orows


def _emit_store(nc, yv, chunk):
    tout_flat, tout, g0, out_r0, orows = chunk
    if orows == HO:
        nc.sync.dma_start(
            out=yv[g0:g0 + PT, :, :, :].rearrange("g k h w -> g (k h w)"),
            in_=tout_flat[:])
    else:
        nc.sync.dma_start(
            out=yv[g0:g0 + PT, :, out_r0:out_r0 + orows, :], in_=tout[:])


_DMA_TYPES = ("InstEventSemaphore",)


def _legalize_waits(nc):
    """TRN2 sequencer codegen allows one sync-wait per compute instruction;
    hoist extras into standalone EventSemaphore ops on the same engine."""
    k = 0
    for bb in nc.m.functions[0].blocks:
        new = []
        for ins in bb.instructions:
            si = ins.sync_info
            ow = list(si.on_wait) if (si and si.on_wait) else []
            if len(ow) > 1 and type(ins).__name__ not in _DMA_TYPES:
                for w in ow[:-1]:
                    k += 1
                    new.append(mybir.InstEventSemaphore(
                        name=f"xtrawait-{k}",
                        opcode="EventSemaphore",
                        engine=ins.engine,
                        sync_info=mybir.SyncInfo(on_wait=[w], on_update=[]),
                    ))
                ins.sync_info = mybir.SyncInfo(
                    on_wait=[ow[-1]], on_update=list(si.on_update or []))
            new.append(ins)
        bb.instructions = new


BUFS = 4


def CHUNK_SCHEDULE(t):
    # FIFO interleaving needs no ramp chunking (the store stream is forced
    # its byte share from the start). The tail trick: split a tiny chunk
    # off tile 0 and emit its store as the queue's LAST item -- its compute
    # finished ~90us earlier, so the queue drains to the end gap-free,
    # while the last full tile's store hides its compute wait behind the
    # preceding store's drain time.
    if t == 0:
        return [(0, 56, ""), (56, 8, "held")]
    if t == NT - 1:
        # Short final compute: the only compute the queue can end up
        # waiting on after the last load is this 8-row chunk's (~1 us).
        return [(0, 56, ""), (56, 8, "")]
    return [(0, H, "")]


def build_program():
    nc = bass.Bass()
    x = nc.dram_tensor("x", [BSH, C, H, W], F32, kind="ExternalInput")
    zmask = nc.dram_tensor("zmask", [PT, NT, 4], F32, kind="ExternalInput")
    y = nc.dram_tensor("y", [BSH, C, HO, WO], BF16, kind="ExternalOutput")
    # Group K consecutive channel-images per partition: they are contiguous
    # in both x and y, so per-partition DMA runs are K*16 KiB loads and
    # K*8712 B stores.
    xv = x[:].rearrange("b c h w -> (b c h w)").rearrange(
        "(g k h w) -> g k h w", k=K, h=H, w=W)
    yv = y[:].rearrange("b c h w -> (b c h w)").rearrange(
        "(g k h w) -> g k h w", k=K, h=HO, w=WO)
    # Single DMA queue (SP) for ALL traffic, FIFO-interleaved with a
    # one-chunk lag: L0,L1,S0,L2,S1,...,Llast,S(last-1),Slast. Measured HW
    # behavior: with separate load/store queues, arbitration starves the
    # compute-gated store queue whenever the load queue has descriptors,
    # building a store backlog that stalls the pipeline via buffer
    # backpressure and drains in a slow stuttering tail. FIFO on one queue
    # enforces the byte ratio exactly; a single queue sustains ~424 GB/s.
    # The lag guarantees a store's compute finished long before the queue
    # reaches it (no head-of-line stall).
    with tile.TileContext(nc) as tc:
        with tc.tile_pool(name="io", bufs=BUFS) as pool:
            with tc.tile_pool(name="zm", bufs=1) as zpool:
                zm = zpool.tile([PT, NT, 4], F32, tag="zm", name="zm")
                nc.scalar.dma_start(out=zm[:], in_=zmask[:])
                pending, held = [], []
                for t in range(NT):
                    for r0, n, tag in CHUNK_SCHEDULE(t):
                        pending.append((tag, _emit_load_compute(
                            nc, pool, xv, yv, zm, t, r0, n, tag=tag)))
                        while len(pending) > 1:
                            tag0, chunk = pending.pop(0)
                            if tag0:
                                held.append(chunk)
                            else:
                                _emit_store(nc, yv, chunk)
                                break
                for _, chunk in pending:
                    _emit_store(nc, yv, chunk)
                for chunk in held:
                    _emit_store(nc, yv, chunk)
    _legalize_waits(nc)
    return nc


def _make_zmask() -> np.ndarray:
    zm = np.ones((PT, NT, 4), np.float32)
    for t in range(NT):
        for p in range(PT):
            b = (t * PT + p) * K // C      # patch index 0..15 (j-independent)
            r, c = b // 4, b % 4
            if r == 0:
                zm[p, t, TOP] = 0.0
            if r == 3:
                zm[p, t, BOT] = 0.0
            if c == 0:
                zm[p, t, LEFT] = 0.0
            if c == 3:
                zm[p, t, RIGHT] = 0.0
    return zm


_NC = None


def _get_nc():
    global _NC
    if _NC is None:
        _NC = build_program()
    return _NC


def kernel(x: np.ndarray) -> np.ndarray:
    assert x.shape == (NCORES * BSH, C, H, W), x.shape
    nc = _get_nc()
    zm = _make_zmask()
    in_maps = [
        {"x": np.ascontiguousarray(x[k * BSH:(k + 1) * BSH]), "zmask": zm}
        for k in range(NCORES)
    ]
    res = run_bass_kernel_spmd(nc, in_maps, list(range(NCORES)))
    # Device output is bf16 (halves store-side HBM traffic; rounding happens
    # after the f32 boundary means, so max rel err ~2^-9). Upcast on host.
    return np.concatenate(
        [np.asarray(r["y"]).astype(np.float32) for r in res.results], axis=0)


# revision 35
# speedup vs baseline: 1.0102x; 1.0102x over previous
"""Trainium2 Bass kernel for nn_Mean_2px_Pad2d.

Full input x: [128, 96, 64, 64] f32.  Output: [128, 96, 66, 66] f32:
  - interior = x
  - borders  = edge-replicate pad, with top/bot rows (cols 1..64) and
    left/right cols (rows 1..64) overwritten by 2-pixel boundary means
  - patches on the image boundary (P=4 grid, 16 patches per image) get
    their outer border row/col zeroed (full 66 length incl. corners)

Sharding: batch 128 = 8 images x 16 patches; one image (16 consecutive
batch entries) per NeuronCore -> identical SPMD program on 8 cores.

Perf design (measured on HW, not theorized):
  - Device output is bf16, upcast to f32 on the host: rounding happens
    AFTER the f32 boundary means so max rel err ~2^-9, far inside the
    2e-2 gate, and it halves store-side HBM traffic.
  - ALL x/y DMA runs on ONE queue (SP HWDGE), FIFO-interleaved with a
    one-chunk lag: L0,L1,S0,L2,S1,...  Measured HW behavior: with
    separate load/store queues, arbitration starves the compute-gated
    store queue whenever the load queue has descriptors ready, building
    a store backlog that stalls the pipeline through buffer
    backpressure and then drains in a slow stuttering tail. FIFO on a
    single queue enforces the byte ratio exactly, one queue sustains
    ~424 GB/s (microbenchmarked; a 2nd adds nothing, a 3rd costs ~15%),
    and per-core the 16 DMA engines run ~100% busy. All 8 cores
    together saturate chip HBM at ~3 TB/s.
  - K=3 CONSECUTIVE channel-images per partition: they are contiguous
    in both x and y, so per-partition DMA runs are 49152 B loads (just
    under the 64 KiB descriptor cap) and 26136 B stores -- fewer
    descriptors, less per-packet overhead (~76 ns each). The 73.6 KiB
    per-partition buffer pair doesn't fit 3 symmetric double buffers;
    asymmetric tin bufs=2 / tout bufs=3 does fit and keeps the
    pipeline deep enough (symmetric bufs=2 serialized the tail ~10 us).
  - 96 channel-images per patch divides by K=3, so an image triple
    never straddles a patch boundary; boundary-patch zeroing is a per-
    partition 0/1 mask multiply (patch intervals in partition space do
    not start on the 0/32/64/96 bases compute ops require). The masks
    ([128] x 4 mega-tiles x 4 edges) are a host-precomputed constant
    table DMA'd once at start on the otherwise-idle ACT queue.
  - Tail structure: a tiny 8-row chunk split off tile 0 has its store
    emitted as the queue's LAST item (computed ~90 us earlier, so the
    queue never ends waiting on compute -- the tile scheduler can
    reorder ready stores ahead of the last load, leaving only
    compute-gated work at the end otherwise), and the last tile is
    split 48+16 so the final compute on the critical path is ~2 us.
"""

import sys

import numpy as np

try:
    import concourse.bass as bass
except ImportError:
    sys.path.insert(0, "/opt/trn_rl_repo")
    import concourse.bass as bass

import concourse.mybir as mybir
import concourse.tile as tile
from concourse.bass_utils import run_bass_kernel_spmd

F32 = mybir.dt.float32
BF16 = mybir.dt.bfloat16

# Per-core shard shapes (hardcoded; full batch 128 / 8 cores).
BSH = 16          # batch entries (patches) per core = one image
C = 96            # channels
H = W = 64
HO = WO = 66      # padded output
G = BSH * C       # 1536 channel-images per core
PT = 128          # partitions per tile
K = 3             # channel-images per partition: 49152 B load runs (just
                  # under the 64 KiB descriptor cap), 26136 B store runs
NT = G // (PT * K)  # 4 mega-tiles
NCORES = 8

TOP, BOT, LEFT, RIGHT = range(4)


def _emit_load_compute(nc, pool, xv, yv, zm, t, r0, n, tag=""):
    """Rows [r0, r0+n) of all K images of a [128, K-image] mega-tile:
    input rows r0..r0+n-1 -> output rows (r0+1)..(r0+n), plus the top
    border row if r0==0 and the bottom border row if r0+n==H, plus
    left/right border cols and boundary-patch zero masks. Returns the
    finished tout for a later (lagged) store emission."""
    g0 = t * PT                                            # image-pair index
    first = r0 == 0
    last = r0 + n == H
    orows = n + (1 if first else 0) + (1 if last else 0)   # output rows
    out_r0 = 0 if first else r0 + 1                        # global out row
    i0 = 1 if first else 0                                 # local 1st interior

    if tag:
        # Dedicated single-buffer slot: its store is held back to the very
        # end of the queue, so it must not block main-tag slot rotation.
        tin_flat = pool.tile([PT, K * n * W], F32,
                             tag=f"tin{tag}", name="tin_flat", bufs=1)
        tout_flat = pool.tile([PT, K * orows * WO], BF16,
                              tag=f"tout{tag}", name="tout_flat", bufs=1)
        tin = tin_flat.rearrange("p (k h w) -> p k h w", k=K, h=n, w=W)
        tout = tout_flat.rearrange("p (k h w) -> p k h w", k=K, h=orows, w=WO)
    else:
        # Asymmetric buffering: tin needs only (loading + computing) = 2
        # slots, tout needs (computing + 2 draining) = 3; together they
        # just fit SBUF at K=3. Symmetric bufs=2 (the naive fit) serializes
        # the tail ~10 us; bufs=2/3 keeps the pipeline deep enough.
        tin_flat = pool.tile([PT, K * H * W], F32, tag="tin",
                             name="tin_flat", bufs=2)
        tout_flat = pool.tile([PT, K * HO * WO], BF16, tag="tout",
                              name="tout_flat", bufs=3)
        tin = tin_flat.rearrange(
            "p (k h w) -> p k h w", k=K, h=H, w=W)[:, :, :n, :]
        tout = tout_flat.rearrange(
            "p (k h w) -> p k h w", k=K, h=HO, w=WO)[:, :, :orows, :]

    # Full-height chunks DMA through the flat 2D view: one contiguous
    # 49152 B (load) / 26136 B (store) run per partition. 4D APs do not
    # coalesce the image dim and would split each run K ways.
    if n == H:
        nc.sync.dma_start(
            out=tin_flat[:],
            in_=xv[g0:g0 + PT, :, :, :].rearrange("g k h w -> g (k h w)"))
    else:
        nc.sync.dma_start(out=tin[:], in_=xv[g0:g0 + PT, :, r0:r0 + n, :])

    # Dummy first write to tout (overwritten below): absorbs the slot-reuse
    # WAR wait so no later compute op carries two semaphore waits (TRN2
    # codegen allows a single sync-wait per instruction).
    nc.vector.memset(tout[:, 0, 0, 0:WO:WO - 1], 0.0)

    # Interior rows
    nc.vector.tensor_copy(tout[:, :, i0:i0 + n, 1:W + 1], tin[:])

    # Border rows (2-px means) + corners (edge-replicate)
    for br, (ra, rb) in (
        ([(0, (0, 1))] if first else []) +
        ([(orows - 1, (n - 2, n - 1))] if last else [])
    ):
        nc.vector.tensor_add(
            tout[:, :, br, 1:W + 1], tin[:, :, ra, :], tin[:, :, rb, :])
        nc.vector.tensor_scalar_mul(
            tout[:, :, br, 1:W + 1], tout[:, :, br, 1:W + 1], 0.5)
        rc = 0 if br == 0 else n - 1
        nc.vector.tensor_copy(
            tout[:, :, br, 0:WO:WO - 1], tin[:, :, rc, 0:W:W - 1])

    # Left+right border cols for this chunk's interior rows
    nc.vector.tensor_add(
        tout[:, :, i0:i0 + n, 0:WO:WO - 1],
        tin[:, :, :, 0:W:W - 2],
        tin[:, :, :, 1:W:W - 2],
    )
    nc.vector.tensor_scalar_mul(
        tout[:, :, i0:i0 + n, 0:WO:WO - 1],
        tout[:, :, i0:i0 + n, 0:WO:WO - 1], 0.5,
    )

    # Zero the outer border of boundary patches: multiply by per-partition
    # 0/1 masks (all K images of a partition share one patch, see header).
    if first:
        nc.vector.tensor_scalar_mul(
            tout[:, :, 0, :], tout[:, :, 0, :], zm[:, t, TOP:TOP + 1])
    if last:
        nc.vector.tensor_scalar_mul(
            tout[:, :, orows - 1, :], tout[:, :, orows - 1, :], zm[:, t, BOT:BOT + 1])
    nc.vector.tensor_scalar_mul(
        tout[:, :, :, 0], tout[:, :, :, 0], zm[:, t, LEFT:LEFT + 1])
    nc.vector.tensor_scalar_mul(
        tout[:, :, :, WO - 1], tout[:, :, :, WO - 1], zm[:, t, RIGHT:RIGHT + 1])

    return tout_flat, tout, g0, out_r0, orows


def _emit_store(nc, yv, chunk):
    tout_flat, tout, g0, out_r0, orows = chunk
    if orows == HO:
        nc.sync.dma_start(
            out=yv[g0:g0 + PT, :, :, :].rearrange("g k h w -> g (k h w)"),
            in_=tout_flat[:])
    else:
        nc.sync.dma_start(
            out=yv[g0:g0 + PT, :, out_r0:out_r0 + orows, :], in_=tout[:])


_DMA_TYPES = ("InstEventSemaphore",)


def _legalize_waits(nc):
    """TRN2 sequencer codegen allows one sync-wait per compute instruction;
    hoist extras into standalone EventSemaphore ops on the same engine."""
    k = 0
    for bb in nc.m.functions[0].blocks:
        new = []
        for ins in bb.instructions:
            si = ins.sync_info
            ow = list(si.on_wait) if (si and si.on_wait) else []
            if len(ow) > 1 and type(ins).__name__ not in _DMA_TYPES:
                for w in ow[:-1]:
                    k += 1
                    new.append(mybir.InstEventSemaphore(
                        name=f"xtrawait-{k}",
                        opcode="EventSemaphore",
                        engine=ins.engine,
                        sync_info=mybir.SyncInfo(on_wait=[w], on_update=[]),
                    ))
                ins.sync_info = mybir.SyncInfo(
                    on_wait=[ow[-1]], on_update=list(si.on_update or []))
            new.append(ins)
        bb.instructions = new


BUFS = 4


def CHUNK_SCHEDULE(t):
    # FIFO interleaving needs no ramp chunking (the store stream is forced
    # its byte share from the start). The tail trick: split a tiny chunk
    # off tile 0 and emit its store as the queue's LAST item -- its compute
    # finished ~90us earlier, so the queue drains to the end gap-free,
    # while the last full tile's store hides its compute wait behind the
    # preceding store's drain time.
    if t == 0:
        return [(0, 56, ""), (56, 8, "held")]
    if t == NT - 1:
        # Short final compute: the only compute the queue can end up
        # waiting on after the last load is this 16-row chunk's (~2 us).
        return [(0, 48, ""), (48, 16, "")]
    return [(0, H, "")]


def build_program():
    nc = bass.Bass()
    x = nc.dram_tensor("x", [BSH, C, H, W], F32, kind="ExternalInput")
    zmask = nc.dram_tensor("zmask", [PT, NT, 4], F32, kind="ExternalInput")
    y = nc.dram_tensor("y", [BSH, C, HO, WO], BF16, kind="ExternalOutput")
    # Group K consecutive channel-images per partition: they are contiguous
    # in both x and y, so per-partition DMA runs are K*16 KiB loads and
    # K*8712 B stores.
    xv = x[:].rearrange("b c h w -> (b c h w)").rearrange(
        "(g k h w) -> g k h w", k=K, h=H, w=W)
    yv = y[:].rearrange("b c h w -> (b c h w)").rearrange(
        "(g k h w) -> g k h w", k=K, h=HO, w=WO)
    # Single DMA queue (SP) for ALL traffic, FIFO-interleaved with a
    # one-chunk lag: L0,L1,S0,L2,S1,...,Llast,S(last-1),Slast. Measured HW
    # behavior: with separate load/store queues, arbitration starves the
    # compute-gated store queue whenever the load queue has descriptors,
    # building a store backlog that stalls the pipeline via buffer
    # backpressure and drains in a slow stuttering tail. FIFO on one queue
    # enforces the byte ratio exactly; a single queue sustains ~424 GB/s.
    # The lag guarantees a store's compute finished long before the queue
    # reaches it (no head-of-line stall).
    with tile.TileContext(nc) as tc:
        with tc.tile_pool(name="io", bufs=BUFS) as pool:
            with tc.tile_pool(name="zm", bufs=1) as zpool:
                zm = zpool.tile([PT, NT, 4], F32, tag="zm", name="zm")
                nc.scalar.dma_start(out=zm[:], in_=zmask[:])
                pending, held = [], []
                for t in range(NT):
                    for r0, n, tag in CHUNK_SCHEDULE(t):
                        pending.append((tag, _emit_load_compute(
                            nc, pool, xv, yv, zm, t, r0, n, tag=tag)))
                        while len(pending) > 1:
                            tag0, chunk = pending.pop(0)
                            if tag0:
                                held.append(chunk)
                            else:
                                _emit_store(nc, yv, chunk)
                                break
                for _, chunk in pending:
                    _emit_store(nc, yv, chunk)
                for chunk in held:
                    _emit_store(nc, yv, chunk)
    _legalize_waits(nc)
    return nc


def _make_zmask() -> np.ndarray:
    zm = np.ones((PT, NT, 4), np.float32)
    for t in range(NT):
        for p in range(PT):
            b = (t * PT + p) * K // C      # patch index 0..15 (j-independent)
            r, c = b // 4, b % 4
            if r == 0:
                zm[p, t, TOP] = 0.0
            if r == 3:
                zm[p, t, BOT] = 0.0
            if c == 0:
                zm[p, t, LEFT] = 0.0
            if c == 3:
                zm[p, t, RIGHT] = 0.0
    return zm


_NC = None


def _get_nc():
    global _NC
    if _NC is None:
        _NC = build_program()
    return _NC


def kernel(x: np.ndarray) -> np.ndarray:
    assert x.shape == (NCORES * BSH, C, H, W), x.shape
    nc = _get_nc()
    zm = _make_zmask()
    in_maps = [
        {"x": np.ascontiguousarray(x[k * BSH:(k + 1) * BSH]), "zmask": zm}
        for k in range(NCORES)
    ]
    res = run_bass_kernel_spmd(nc, in_maps, list(range(NCORES)))
    # Device output is bf16 (halves store-side HBM traffic; rounding happens
    # after the f32 boundary means, so max rel err ~2^-9). Upcast on host.
    return np.concatenate(
        [np.asarray(r["y"]).astype(np.float32) for r in res.results], axis=0)
